# revision 25
# baseline (speedup 1.0000x reference)
"""DynamicKGE (RGCN message passing) Trainium2 kernel.

Math (see reference):
  conv[n] = sum_r ( sum_{e: dst_e=n, type_e=r} norm_e * x[src_e] ) @ W[r] + x[n] @ root
  entity_o = ge*e_emb + (1-ge)*relu(conv),   ge = sigmoid(gate_entity)
  relation_o = gr*relation_emb + (1-ge)*relation_ctx   (replicated gate bug)

Strategy: shard destination nodes across 8 cores (no collectives). Per core,
edges are grouped by (node-tile of 512 dst, relation); each 128-edge chunk is
gathered (fp16 rows) with dma_gather — the x table is split into lo/hi halves
so row ids fit int16 — then a one-hot selection matrix
P[e, j] = norm_e * (dstoff_e == j) is built with one fused DVE
tensor_scalar(is_equal, mult) against a replicated iota, and PE matmuls do
   M1_T[d, 512] += Xg[e, d].T @ P[e, 512]     (scatter + segment-sum)
   outT[d2, 512] += W_s[d, d2].T @ M1_T       (grouped GEMM)
with the root term a direct matmul against a host-pretransposed own-x tile.
Program structure (chunk counts) is the max over cores so one NEFF runs SPMD
on all 8 cores; cores pad short slices with norm=0 edges. Outputs are
produced transposed ([d, n]) and un-transposed on the host.
"""

import math
from contextlib import ExitStack
from dataclasses import dataclass, field

import numpy as np


# ---------------------------------------------------------------- config

@dataclass
class Cfg:
    N: int = 50000          # entities
    NR: int = 8             # relations (relation_emb rows)
    R: int = 16             # relation types (2*NR)
    D: int = 128            # dim
    NC: int = 8             # cores
    TILE_N: int = 448       # dst nodes per PSUM tile
    GB: int = 32            # gather batch, in 128-edge chunks

    @property
    def NPC(self):  # nodes per core
        return math.ceil(self.N / self.NC)

    @property
    def NTILES(self):
        return math.ceil(self.NPC / self.TILE_N)

    @property
    def HALF(self):  # lo/hi split of the gather table (int16 row ids)
        return (self.N + 1) // 2


# ---------------------------------------------------------------- host prep

@dataclass
class Prep:
    K: list                     # [NTILES][R][2] chunk counts (max over cores)
    C: tuple                    # (C_lo, C_hi) total chunks per stream
    in_maps: list = field(default_factory=list)


def _host_prep(cfg: Cfg, entity, edge_index, edge_type, edge_norm,
               entity_emb, relation_emb, entity_ctx, relation_ctx,
               gate_entity, gate_relation, conv1_weight_rel, conv1_root):
    N, R, D, NC = cfg.N, cfg.R, cfg.D, cfg.NC
    NPC, TILE_N, NTILES = cfg.NPC, cfg.TILE_N, cfg.NTILES
    HALF = cfg.HALF
    assert HALF <= 32768

    entity = np.asarray(entity).astype(np.int64)
    dst = np.asarray(edge_index[0]).astype(np.int64)
    src = np.asarray(edge_index[1]).astype(np.int64)
    et = np.asarray(edge_type).astype(np.int64)
    nrm = np.asarray(edge_norm).astype(np.float32)

    e_emb = np.asarray(entity_emb, dtype=np.float32)[entity]   # [N, D]
    e_ctx = np.asarray(entity_ctx, dtype=np.float32)[entity]   # [N, D]
    x16 = e_ctx.astype(np.float16)                             # gather table
    x_lo, x_hi = x16[:HALF], x16[HALF:]
    n_hi = N - HALF

    core = dst // NPC
    loc = dst - core * NPC
    tilei = loc // TILE_N
    dstoff = loc - tilei * TILE_N
    half = (src >= HALF).astype(np.int64)

    # group edges by (core, tile, type, half); counts -> shared chunk table
    gkey = ((core * NTILES + tilei) * R + et) * 2 + half
    order = np.argsort(gkey, kind="stable")
    counts = np.bincount(gkey, minlength=NC * NTILES * R * 2) \
        .reshape(NC, NTILES, R, 2)
    K = -(-counts.max(axis=0) // 128)                 # [NTILES, R, 2]

    # stream-local chunk offsets: lo chunks and hi chunks each form a stream,
    # ordered by (tile, type)
    base = np.zeros((NTILES, R, 2), dtype=np.int64)
    base[..., 0].reshape(-1)[1:] = np.cumsum(K[..., 0].reshape(-1))[:-1]
    base[..., 1].reshape(-1)[1:] = np.cumsum(K[..., 1].reshape(-1))[:-1]
    C_lo = int(K[..., 0].sum())
    C_hi = int(K[..., 1].sum())

    # per-edge slot position within its stream
    gstart = np.zeros(NC * NTILES * R * 2, dtype=np.int64)
    csum = np.cumsum(counts.reshape(-1))
    gstart[1:] = csum[:-1]
    E = dst.shape[0]
    rank = np.empty(E, dtype=np.int64)
    rank[order] = np.arange(E) - gstart[gkey[order]]
    pos = base[tilei, et, half] * 128 + rank

    idx_s = [np.zeros((NC, C_lo * 128), np.int16),
             np.zeros((NC, C_hi * 128), np.int16)]
    off_s = [np.full((NC, C_lo * 128), -1.0, np.float32),
             np.full((NC, C_hi * 128), -1.0, np.float32)]
    nrm_s = [np.zeros((NC, C_lo * 128), np.float32),
             np.zeros((NC, C_hi * 128), np.float32)]
    for h in (0, 1):
        m = half == h
        idx_s[h][core[m], pos[m]] = (src[m] - h * HALF).astype(np.int16)
        off_s[h][core[m], pos[m]] = dstoff[m]
        nrm_s[h][core[m], pos[m]] = nrm[m]

    # dma_gather int16 index packing: idx i of a batch at [i%16, i//16],
    # replicated across the 8 16-partition groups
    GB = cfg.GB

    def pack_idx(a, C):
        # a: [C*128] int16 -> [128, C*8]
        out = np.zeros((128, C * 8), np.int16)
        g = 0
        while g < C:
            nb = min(GB, C - g)
            ids = a[g * 128:(g + nb) * 128]
            arr = ids.reshape(nb * 8, 16).T        # [16, nb*8]
            out[:, g * 8:(g + nb) * 8] = np.tile(arr, (8, 1))
            g += nb
        return out

    def to_sb(a, C):
        return np.ascontiguousarray(a.reshape(C, 128).T)

    # weights: slots 0..R-1 = W_r, slot R = root ; packed [128, (R+1)*128] fp16
    w = np.asarray(conv1_weight_rel, dtype=np.float32)
    root = np.asarray(conv1_root, dtype=np.float32)
    w_sb = np.concatenate([w, root[None]], axis=0)    # [R+1, D, D]
    w_sb = np.ascontiguousarray(w_sb.transpose(1, 0, 2).reshape(D, (R + 1) * D))
    w_sb = w_sb.astype(np.float16)

    iota = np.tile(np.arange(TILE_N, dtype=np.float16)[None, :], (128, 1))

    rel_emb_t = np.ascontiguousarray(np.asarray(relation_emb, np.float32).T)
    rel_ctx_t = np.ascontiguousarray(np.asarray(relation_ctx, np.float32).T)
    gate_e = np.asarray(gate_entity, np.float32).reshape(D, 1)
    gate_r = np.asarray(gate_relation, np.float32).reshape(D, 1)

    prep = Prep(K=K.tolist(), C=(C_lo, C_hi))
    for c in range(NC):
        lo = c * NPC
        n_real = min(NPC, N - lo)
        emb_pad = np.zeros((NTILES * TILE_N, D), np.float16)
        emb_pad[:n_real] = e_emb[lo:lo + n_real]
        own_pad = np.zeros((NTILES * TILE_N, D), np.float16)
        own_pad[:n_real] = x16[lo:lo + n_real]
        prep.in_maps.append({
            "x_lo": x_lo,
            "x_hi": x_hi,
            "xown_t": np.ascontiguousarray(own_pad.T),
            "emb_t": np.ascontiguousarray(emb_pad.T),
            "idx_lo": pack_idx(idx_s[0][c], C_lo),
            "idx_hi": pack_idx(idx_s[1][c], C_hi),
            "doff_lo": to_sb(off_s[0][c], C_lo),
            "doff_hi": to_sb(off_s[1][c], C_hi),
            "nrm_lo": to_sb(nrm_s[0][c], C_lo),
            "nrm_hi": to_sb(nrm_s[1][c], C_hi),
            "w16": w_sb,
            "iota": iota,
            "gate_e": gate_e,
            "gate_r": gate_r,
            "rel_emb_t": rel_emb_t,
            "rel_ctx_t": rel_ctx_t,
        })
    return prep, (HALF, n_hi)


# ---------------------------------------------------------------- program

ABLATE = {"gather": True, "mm": True, "p": True, "ntiles": None}
TUNE = {"p_bufs": 6, "xg_bufs": 3, "m1_bufs": 4, "ps_bufs": 3}


def _build_program(cfg: Cfg, K, C, halves):
    import concourse.mybir as mybir
    import concourse.tile as tile
    from concourse import bacc, library_config

    N, D, TILE_N, NTILES, R, NR = cfg.N, cfg.D, cfg.TILE_N, cfg.NTILES, cfg.R, cfg.NR
    f16, f32, i16 = mybir.dt.float16, mybir.dt.float32, mybir.dt.int16
    AF = mybir.ActivationFunctionType
    OP = mybir.AluOpType
    GB = cfg.GB
    C_lo, C_hi = C
    HALF, n_hi = halves

    nc = bacc.Bacc("TRN2", target_bir_lowering=False, debug=False,
                   num_devices=cfg.NC, num_swdge_queues=2)

    x_lo = nc.dram_tensor("x_lo", [HALF, D], f16, kind="ExternalInput").ap()
    x_hi = nc.dram_tensor("x_hi", [n_hi, D], f16, kind="ExternalInput").ap()
    xown_t = nc.dram_tensor("xown_t", [D, NTILES * TILE_N], f16, kind="ExternalInput").ap()
    emb_t = nc.dram_tensor("emb_t", [D, NTILES * TILE_N], f16, kind="ExternalInput").ap()
    idx_d = [nc.dram_tensor("idx_lo", [128, max(C_lo, 1) * 8], i16, kind="ExternalInput").ap(),
             nc.dram_tensor("idx_hi", [128, max(C_hi, 1) * 8], i16, kind="ExternalInput").ap()]
    doff_d = [nc.dram_tensor("doff_lo", [128, max(C_lo, 1)], f32, kind="ExternalInput").ap(),
              nc.dram_tensor("doff_hi", [128, max(C_hi, 1)], f32, kind="ExternalInput").ap()]
    nrm_d = [nc.dram_tensor("nrm_lo", [128, max(C_lo, 1)], f32, kind="ExternalInput").ap(),
             nc.dram_tensor("nrm_hi", [128, max(C_hi, 1)], f32, kind="ExternalInput").ap()]
    w16 = nc.dram_tensor("w16", [D, (R + 1) * D], f16, kind="ExternalInput").ap()
    iota_d = nc.dram_tensor("iota", [128, TILE_N], f16, kind="ExternalInput").ap()
    gate_e = nc.dram_tensor("gate_e", [D, 1], f32, kind="ExternalInput").ap()
    gate_r = nc.dram_tensor("gate_r", [D, 1], f32, kind="ExternalInput").ap()
    rel_emb_t = nc.dram_tensor("rel_emb_t", [D, NR], f32, kind="ExternalInput").ap()
    rel_ctx_t = nc.dram_tensor("rel_ctx_t", [D, NR], f32, kind="ExternalInput").ap()

    out_t = nc.dram_tensor("out_t", [D, NTILES * TILE_N], f32, kind="ExternalOutput").ap()
    rel_out = nc.dram_tensor("rel_out_t", [D, NR], f32, kind="ExternalOutput").ap()

    xsrc = [x_lo, x_hi]

    with ExitStack() as ctx:
        tc = ctx.enter_context(tile.TileContext(nc))
        const = ctx.enter_context(tc.tile_pool(name="const", bufs=1))
        xgp = [ctx.enter_context(tc.tile_pool(name="xg_lo", bufs=TUNE["xg_bufs"])),
               ctx.enter_context(tc.tile_pool(name="xg_hi", bufs=TUNE["xg_bufs"]))]
        pp = ctx.enter_context(tc.tile_pool(name="p", bufs=TUNE["p_bufs"]))
        m1p = ctx.enter_context(tc.tile_pool(name="m1sb", bufs=TUNE["m1_bufs"]))
        outp = ctx.enter_context(tc.tile_pool(name="outsb", bufs=2))
        embp = ctx.enter_context(tc.tile_pool(name="embt", bufs=2))
        ownp = ctx.enter_context(tc.tile_pool(name="ownt", bufs=2))
        ps_m1 = ctx.enter_context(tc.tile_pool(name="psm1", bufs=TUNE["ps_bufs"], space="PSUM"))
        ps_out = ctx.enter_context(tc.tile_pool(name="psout", bufs=2, space="PSUM"))

        # dma_gather lives in the mlp ext-isa Q7 library
        nc.gpsimd.load_library(library_config.mlp)

        # constants
        iota_sb = const.tile([128, TILE_N], f16)
        nc.sync.dma_start(iota_sb[:], iota_d[:])
        w_sb = const.tile([D, (R + 1) * D], f16)
        nc.sync.dma_start(w_sb[:], w16[:])
        idx_sb, doff_sb, nrm_sb = [], [], []
        for h, Ch in ((0, C_lo), (1, C_hi)):
            t_ = const.tile([128, max(Ch, 1) * 8], i16, tag=f"idx{h}")
            nc.sync.dma_start(t_[:], idx_d[h][:])
            idx_sb.append(t_)
            t_ = const.tile([128, max(Ch, 1)], f32, tag=f"doff{h}")
            nc.sync.dma_start(t_[:], doff_d[h][:])
            doff_sb.append(t_)
            t_ = const.tile([128, max(Ch, 1)], f32, tag=f"nrm{h}")
            nc.sync.dma_start(t_[:], nrm_d[h][:])
            nrm_sb.append(t_)

        # gates: sigmoid via exp to keep precision
        def sigmoid_cols(gate_ap, pfx):
            g = const.tile([D, 1], f32, tag=f"{pfx}_g")
            nc.sync.dma_start(g[:], gate_ap[:])
            en = const.tile([D, 1], f32, tag=f"{pfx}_en")
            nc.scalar.activation(out=en[:], in_=g[:], func=AF.Exp, scale=-1.0)
            den = const.tile([D, 1], f32, tag=f"{pfx}_den")
            nc.vector.tensor_scalar(out=den[:], in0=en[:], scalar1=1.0,
                                    scalar2=None, op0=OP.add)
            sg = const.tile([D, 1], f32, tag=f"{pfx}_sg")
            nc.vector.reciprocal(out=sg[:], in_=den[:])
            om = const.tile([D, 1], f32, tag=f"{pfx}_om")
            nc.vector.tensor_scalar(out=om[:], in0=sg[:], scalar1=-1.0,
                                    scalar2=1.0, op0=OP.mult, op1=OP.add)
            return sg, om

        ge, one_m_ge = sigmoid_cols(gate_e, "ge")
        gr, _ = sigmoid_cols(gate_r, "gr")

        # relation output (tiny)
        re_sb = const.tile([D, NR], f32, tag="re_sb")
        nc.sync.dma_start(re_sb[:], rel_emb_t[:])
        rc_sb = const.tile([D, NR], f32, tag="rc_sb")
        nc.sync.dma_start(rc_sb[:], rel_ctx_t[:])
        r1 = const.tile([D, NR], f32, tag="r1")
        nc.vector.tensor_scalar(out=r1[:], in0=re_sb[:], scalar1=gr[:, 0:1],
                                scalar2=None, op0=OP.mult)
        r2 = const.tile([D, NR], f32, tag="r2")
        nc.vector.tensor_scalar(out=r2[:], in0=rc_sb[:], scalar1=one_m_ge[:, 0:1],
                                scalar2=None, op0=OP.mult)
        r3 = const.tile([D, NR], f32, tag="r3")
        nc.vector.tensor_tensor(out=r3[:], in0=r1[:], in1=r2[:], op=OP.add)
        nc.sync.dma_start(rel_out[:], r3[:])

        # gather streams: g[h] = next chunk to consume; issue batches of GB
        gcur = [0, 0]
        gbase = [0, 0]
        xg = [None, None]
        Ctot = [C_lo, C_hi]

        def next_chunk(h):
            g = gcur[h]
            if xg[h] is None or g - gbase[h] >= GB:
                nb = min(GB, Ctot[h] - g)
                xg[h] = xgp[h].tile([128, nb * D], f16, tag=f"xg{h}", name=f"xg{h}_{g}")
                if ABLATE["gather"]:
                    nc.gpsimd.dma_gather(
                        out_ap=xg[h][:].rearrange("p (b d) -> p b d", d=D),
                        in_ap=xsrc[h][:, :],
                        idxs_ap=idx_sb[h][:, g * 8:(g + nb) * 8],
                        num_idxs=nb * 128,
                        num_idxs_reg=nb * 128,
                        elem_size=D,
                        single_packet=False,
                        queue_num=h,
                    )
                else:
                    nc.vector.memset(xg[h][:], 1.0)
                gbase[h] = g
            gcur[h] = g + 1
            return xg[h][:, (g - gbase[h]) * D:(g - gbase[h] + 1) * D], g

        ntiles_run = NTILES if ABLATE["ntiles"] is None else min(
            NTILES, ABLATE["ntiles"])
        for t in range(ntiles_run):
            out_ps = ps_out.tile([D, TILE_N], mybir.dt.float32, tag="psout")
            first_mm2 = True
            for s in range(R):
                kk = [K[t][s][0], K[t][s][1]]
                ktot = kk[0] + kk[1]
                if ktot == 0:
                    continue
                m1_ps = ps_m1.tile([D, TILE_N], mybir.dt.float32, tag="psm1")
                ci = 0
                for h in (0, 1):
                    for _ in range(kk[h]):
                        lhs, g = next_chunk(h)
                        if ABLATE["p"]:
                            p_t = pp.tile([128, TILE_N], f16, tag="p")
                            nc.vector.tensor_scalar(
                                out=p_t[:], in0=iota_sb[:],
                                scalar1=doff_sb[h][:, g:g + 1],
                                scalar2=nrm_sb[h][:, g:g + 1],
                                op0=OP.is_equal, op1=OP.mult)
                        else:
                            p_t = iota_sb  # stale data, timing only
                        if ABLATE["mm"] or ci == 0:
                            nc.tensor.matmul(
                                out=m1_ps[:], lhsT=lhs, rhs=p_t[:],
                                start=(ci == 0),
                                stop=(ci == ktot - 1) if ABLATE["mm"] else True)
                        ci += 1
                m1_sb = m1p.tile([D, TILE_N], f16, tag="m1sb")
                nc.scalar.activation(out=m1_sb[:], in_=m1_ps[:], func=AF.Copy)
                nc.tensor.matmul(
                    out=out_ps[:], lhsT=w_sb[:, s * D:(s + 1) * D], rhs=m1_sb[:],
                    start=first_mm2, stop=False)
                first_mm2 = False
            # root term: outT += root.T @ xownT_tile  (root at slot R)
            own = ownp.tile([D, TILE_N], f16, tag="ownt")
            nc.sync.dma_start(own[:], xown_t[:, t * TILE_N:(t + 1) * TILE_N])
            nc.tensor.matmul(out=out_ps[:], lhsT=w_sb[:, R * D:(R + 1) * D],
                             rhs=own[:], start=first_mm2, stop=True)
            # epilogue: out = (1-ge)*relu(conv) + ge*e_emb
            o_sb = outp.tile([D, TILE_N], f32, tag="outsb")
            nc.scalar.activation(out=o_sb[:], in_=out_ps[:], func=AF.Relu,
                                 scale=one_m_ge[:, 0:1])
            em = embp.tile([D, TILE_N], f16, tag="embt")
            nc.sync.dma_start(em[:], emb_t[:, t * TILE_N:(t + 1) * TILE_N])
            em2 = embp.tile([D, TILE_N], f32, tag="embt2")
            nc.vector.tensor_scalar(out=em2[:], in0=em[:], scalar1=ge[:, 0:1],
                                    scalar2=None, op0=OP.mult)
            fin = outp.tile([D, TILE_N], f32, tag="fin")
            nc.vector.tensor_tensor(out=fin[:], in0=o_sb[:], in1=em2[:], op=OP.add)
            nc.sync.dma_start(out_t[:, t * TILE_N:(t + 1) * TILE_N], fin[:])

        if ABLATE["ntiles"] is None:
            assert gcur[0] == C_lo and gcur[1] == C_hi, (gcur, C_lo, C_hi)

    nc.compile()
    return nc


# ---------------------------------------------------------------- entry

TIME_REPEAT = 0      # test harness sets >1 to measure per-iteration device time
timed_ns = None


def _exec_spmd(nc, in_maps, n_cores):
    """Execute the Bass program on n_cores via PJRT/axon.

    Mirrors bass2jax.run_bass_via_pjrt's multi-core path, with an optional
    chained-execute repeat loop (iteration i+1's donated output buffers are
    iteration i's outputs) so per-iteration device time can be measured with
    a single dispatch.
    """
    global timed_ns
    import time as _time

    import jax
    from jax.experimental.shard_map import shard_map
    from jax.sharding import Mesh, PartitionSpec

    import concourse.mybir as mybir
    from concourse import bass2jax

    bass2jax.install_neuronx_cc_hook()
    assert nc.dbg_addr is None
    partition_name = (nc.partition_id_tensor.name
                      if nc.partition_id_tensor else None)

    in_names, out_names, out_avals, zero_outs = [], [], [], []
    for alloc in nc.m.functions[0].allocations:
        if not isinstance(alloc, mybir.MemoryLocationSet):
            continue
        name = alloc.memorylocations[0].name
        if alloc.kind == "ExternalInput":
            if name != partition_name:
                in_names.append(name)
        elif alloc.kind == "ExternalOutput":
            out_names.append(name)
            shape = tuple(alloc.tensor_shape)
            dtype = mybir.dt.np(alloc.dtype)
            out_avals.append(jax.core.ShapedArray(shape, dtype))
            zero_outs.append(np.zeros(shape, dtype))
    n_params = len(in_names)
    n_outs = len(out_avals)
    all_names = in_names + out_names
    if partition_name is not None:
        all_names = all_names + [partition_name]
    repeat = max(1, TIME_REPEAT)

    def _body(*args):
        operands = list(args)
        if partition_name is not None:
            operands.append(bass2jax.partition_id_tensor())
        outs = bass2jax._bass_exec_p.bind(
            *operands,
            out_avals=tuple(out_avals),
            in_names=tuple(all_names),
            out_names=tuple(out_names),
            lowering_input_output_aliases=(),
            sim_require_finite=True,
            sim_require_nnan=True,
            nc=nc,
        )
        return tuple(outs)

    devices = jax.devices()[:n_cores]
    assert len(devices) == n_cores
    mesh = Mesh(np.asarray(devices), ("core",))
    donate = tuple(range(n_params, n_params + n_outs))
    sharded = jax.jit(
        shard_map(_body, mesh=mesh,
                  in_specs=(PartitionSpec("core"),) * (n_params + n_outs),
                  out_specs=(PartitionSpec("core"),) * n_outs,
                  check_rep=False),
        donate_argnums=donate, keep_unused=True)

    concat_in = [
        np.concatenate([np.asarray(m[name]) for m in in_maps], axis=0)
        for name in in_names]
    concat_zeros = [
        np.zeros((n_cores * z.shape[0], *z.shape[1:]), z.dtype)
        for z in zero_outs]

    out_arrs = sharded(*concat_in, *concat_zeros)
    jax.block_until_ready(out_arrs)

    if TIME_REPEAT > 1:
        # non-donating jit; inputs pinned on device; N async dispatches
        from jax.sharding import NamedSharding
        nodon = jax.jit(
            shard_map(_body, mesh=mesh,
                      in_specs=(PartitionSpec("core"),) * (n_params + n_outs),
                      out_specs=(PartitionSpec("core"),) * n_outs,
                      check_rep=False),
            keep_unused=True)
        shd = NamedSharding(mesh, PartitionSpec("core"))
        din = [jax.device_put(a, shd) for a in concat_in + concat_zeros]
        jax.block_until_ready(din)
        jax.block_until_ready(nodon(*din))          # warm + compile
        t0 = _time.perf_counter()
        last = None
        for _ in range(repeat):
            last = nodon(*din)
        jax.block_until_ready(last)
        dt = _time.perf_counter() - t0
        timed_ns = dt / repeat * 1e9

    return [
        {name: np.asarray(out_arrs[i]).reshape(n_cores, *out_avals[i].shape)[c]
         for i, name in enumerate(out_names)}
        for c in range(n_cores)]


def _run(cfg: Cfg, inputs: dict):
    prep, halves = _host_prep(cfg, **inputs)
    nc = _build_program(cfg, prep.K, prep.C, halves)
    results = _exec_spmd(nc, prep.in_maps, cfg.NC)
    ent = np.concatenate(
        [r["out_t"][:, :min(cfg.NPC, cfg.N - c * cfg.NPC)].T
         for c, r in enumerate(results)], axis=0).astype(np.float32)
    rel = results[0]["rel_out_t"].T.astype(np.float32)
    return ent, rel


def kernel(entity, edge_index, edge_type, edge_norm, DAD_rel,
           entity_emb, relation_emb, entity_ctx, relation_ctx,
           gate_entity, gate_relation, conv1_weight_rel, conv1_root):
    cfg = Cfg()
    return _run(cfg, dict(
        entity=entity, edge_index=edge_index, edge_type=edge_type,
        edge_norm=edge_norm, entity_emb=entity_emb, relation_emb=relation_emb,
        entity_ctx=entity_ctx, relation_ctx=relation_ctx,
        gate_entity=gate_entity, gate_relation=gate_relation,
        conv1_weight_rel=conv1_weight_rel, conv1_root=conv1_root))
